# revision 1
# baseline (speedup 1.0000x reference)
"""Trainium2 Bass kernel for nn_Attention_8358006358422.

Reference computation (B=64, V=8, D=1024):
    BN over all B*V rows per feature d -> img
    x_qk = qk_w @ img ; x_v = v_w @ img + bias
    energy[b] = x_qk[b]^T x_qk[b]  (D x D, contraction over V)
    att = softmax(energy, -1); att /= (1e-9 + sum(att, axis=1))
    out = img + x_v @ att

Kernel strategy (8 NeuronCores, data-parallel over B, 8 batches/core):
  * BN stats are global -> every core redundantly reduces the full
    feat (2 MB) with ones-vector matmuls (partition reduction).
  * softmax folded algebraically:
        E = exp(energy) (no max-subtraction needed; |energy| < ~40)
        rowsum[d] = sum_e E[d,e]  (free accumulator of the ACT exp op)
        Y = [x_v^T * recip(rowsum) ; recip(rowsum)]^T @ E   (9 x D)
        out = img + Y[0:8] * recip(1e-9 + Y[8])   (column renorm folded)
    so the 1M-element attention matrix is never renormalized
    elementwise - only exp'd once.
  * x_qk/x_v weights are loaded transposed via strided DMA; energy and
    the Y matmul run in float32r (4x PE streaming rate vs fp32), with all
    producers typed f32r as walrus requires.
  * the batch loop is software-pipelined: batch b+1's BN/x_qk/x_v prep
    is emitted inside batch b so strict per-engine FIFOs never stall.
"""

import sys
import numpy as np

sys.path.insert(0, "/opt/trn_rl_repo")

B, V, D = 64, 8, 1024
NCORES = 8
BPC = B // NCORES          # batches per core
ROWS = B * V               # 512 BN rows
SHARD_ROWS = BPC * V       # 64
NBLK = D // 128            # 8 d-blocks of 128
BN_EPS = 1e-5

_BUILT = None


def _build_program():
    import concourse.bass as bass
    import concourse.mybir as mybir
    import concourse.tile as tile
    from concourse import bacc
    from contextlib import ExitStack

    fp32 = mybir.dt.float32
    F32R = mybir.dt.float32r
    MULT = mybir.AluOpType.mult
    ADD = mybir.AluOpType.add
    SUB = mybir.AluOpType.subtract
    EXP = mybir.ActivationFunctionType.Exp
    LN = mybir.ActivationFunctionType.Ln
    SQUARE = mybir.ActivationFunctionType.Square

    nc = bacc.Bacc(
        "TRN2",
        target_bir_lowering=False,
        debug=False,
        enable_asserts=False,
        num_devices=NCORES,
    )

    # ---- DRAM I/O ----
    feat_full = nc.dram_tensor("feat_full", [ROWS, D], F32R, kind="ExternalInput")
    feat_shard = nc.dram_tensor("feat_shard", [SHARD_ROWS, D], F32R, kind="ExternalInput")
    gamma_d = nc.dram_tensor("gamma", [1, D], fp32, kind="ExternalInput")
    beta_d = nc.dram_tensor("beta", [1, D], fp32, kind="ExternalInput")
    qk_d = nc.dram_tensor("qk_w", [V, V], F32R, kind="ExternalInput")
    vw_d = nc.dram_tensor("v_w", [V, V], F32R, kind="ExternalInput")
    vb_d = nc.dram_tensor("v_bias", [1, V], fp32, kind="ExternalInput")
    out_d = nc.dram_tensor("out", [SHARD_ROWS, D], fp32, kind="ExternalOutput")

    with tile.TileContext(nc) as tc, ExitStack() as ctx:
        const = ctx.enter_context(tc.tile_pool(name="const", bufs=1))
        ftp = ctx.enter_context(tc.tile_pool(name="ftp", bufs=4))
        sqp = ctx.enter_context(tc.tile_pool(name="sqp", bufs=2))
        imgp = ctx.enter_context(tc.tile_pool(name="imgp", bufs=3))
        xgp = ctx.enter_context(tc.tile_pool(name="xgp", bufs=3))
        esbp = ctx.enter_context(tc.tile_pool(name="esbp", bufs=4))
        smallp = ctx.enter_context(tc.tile_pool(name="smallp", bufs=6))
        finp = ctx.enter_context(tc.tile_pool(name="finp", bufs=3))
        xvp = ctx.enter_context(tc.tile_pool(name="xvp", bufs=2))

        pe_pool = ctx.enter_context(tc.tile_pool(name="pe", bufs=2, space="PSUM"))
        py_pool = ctx.enter_context(tc.tile_pool(name="py", bufs=1, space="PSUM"))
        paux = ctx.enter_context(tc.tile_pool(name="paux", bufs=2, space="PSUM"))

        # ---- constants ----
        ones_col = const.tile([128, 1], fp32)
        nc.vector.memset(ones_col[:], 1.0)
        ones_col_r = const.tile([128, 1], F32R)
        nc.vector.tensor_scalar_mul(ones_col_r[:], ones_col[:], 1.0)
        ones_row = const.tile([1, 128], fp32)
        nc.vector.memset(ones_row[:], 1.0)

        # ---- BN statistics over all 512 rows (redundant on every core) ----
        ffull = feat_full[:, :]
        sum_ps = pe_pool.tile([1, D], fp32, tag="pe", name="sum_ps")
        sq_ps = pe_pool.tile([1, D], fp32, tag="pe", name="sq_ps")
        ft_tiles = []
        for r in range(4):
            ft = ftp.tile([128, D], F32R)
            nc.sync.dma_start(ft[0:64, :], ffull[128 * r : 128 * r + 64, :])
            nc.sync.dma_start(ft[64:128, :], ffull[128 * r + 64 : 128 * (r + 1), :])
            ft_tiles.append(ft)
        gamma_sb = const.tile([1, D], fp32)
        nc.sync.dma_start(gamma_sb[:], gamma_d[:, :])
        beta_sb = const.tile([1, D], fp32)
        nc.sync.dma_start(beta_sb[:], beta_d[:, :])
        qkT_sb = const.tile([V, V], F32R)
        nc.sync.dma_start(qkT_sb[:], qk_d[:, :].rearrange("o i -> i o"))
        vwT_sb = const.tile([V, V], F32R)
        nc.sync.dma_start(vwT_sb[:], vw_d[:, :].rearrange("o i -> i o"))
        vb_sb = const.tile([1, V], fp32)
        nc.sync.dma_start(vb_sb[:], vb_d[:, :])

        for r in range(4):
            ft = ft_tiles[r]
            sq = sqp.tile([128, D], F32R)
            nc.vector.tensor_mul(sq[:], ft[:], ft[:])
            st = r == 0
            sp = r == 3
            for h in range(2):
                cols = slice(512 * h, 512 * (h + 1))
                nc.tensor.matmul(sum_ps[0:1, cols], ones_col_r[:], ft[:, cols],
                                 start=st, stop=sp)
                nc.tensor.matmul(sq_ps[0:1, cols], ones_col_r[:], sq[:, cols],
                                 start=st, stop=sp)

        # mean, var, alpha=gamma*rstd, beta2=beta-mean*alpha.
        # Run the chain in column halves so each step's latency halves and
        # the two halves pipeline through DVE.
        mean_sb = const.tile([1, D], fp32)
        msq = const.tile([1, D], fp32)
        msq2 = const.tile([1, D], fp32)
        vpe = const.tile([1, D], fp32)
        rv = const.tile([1, D], fp32)
        rstd = const.tile([1, D], fp32)
        alpha_row = const.tile([1, D], fp32)
        tmp_row = const.tile([1, D], fp32)
        beta2_row = const.tile([1, D], fp32)
        alphaB = const.tile([V, D], fp32)
        beta2B = const.tile([V, D], fp32)
        for h in range(2):
            c = slice(512 * h, 512 * (h + 1))
            nc.vector.tensor_scalar_mul(mean_sb[:, c], sum_ps[0:1, c], 1.0 / ROWS)
            nc.vector.tensor_mul(msq[:, c], mean_sb[:, c], mean_sb[:, c])
            nc.vector.tensor_scalar_sub(msq2[:, c], msq[:, c], BN_EPS)
            nc.vector.scalar_tensor_tensor(vpe[:, c], sq_ps[0:1, c], 1.0 / ROWS,
                                           msq2[:, c], op0=MULT, op1=SUB)
            # rstd = (var+eps)^-0.5 via exp(-0.5*ln(.)): Ln and Exp share one
            # ACT table set, so no mid-kernel table switch for a sqrt
            nc.scalar.activation(rv[:, c], vpe[:, c], LN)
            nc.scalar.activation(rstd[:, c], rv[:, c], EXP, scale=-0.5)
            nc.vector.tensor_mul(alpha_row[:, c], gamma_sb[:, c], rstd[:, c])
            nc.vector.tensor_mul(tmp_row[:, c], mean_sb[:, c], alpha_row[:, c])
            nc.vector.tensor_sub(beta2_row[:, c], beta_sb[:, c], tmp_row[:, c])
            nc.gpsimd.partition_broadcast(alphaB[:, c], alpha_row[:, c])
            nc.gpsimd.partition_broadcast(beta2B[:, c], beta2_row[:, c])

        fshard = feat_shard[:, :]
        out_ap = out_d[:, :]

        xaug_ring = []
        for i in range(3):
            xa = const.tile([128, 33], F32R, name=f"xaug_ring{i}")
            nc.vector.tensor_scalar_mul(xa[:, V:32], ft_tiles[0][:, 0 : 32 - V], 0.0)
            xaug_ring.append(xa)

        # ---- main per-batch pipeline (software-pipelined: batch b+1's
        # prep is emitted mid-batch-b so strict per-engine FIFOs don't
        # serialize BN/x_qk/x_v behind batch b's tail) ----
        state = {}

        def prepare(b):
            img = imgp.tile([V, D], F32R, tag="img", name=f"img{b}")
            nc.sync.dma_start(img[:], fshard[V * b : V * (b + 1), :])
            xg_sb = xgp.tile([V, D], F32R, tag="xq", name=f"xq{b}")
            if b == 0:
                # first batch: run BN -> x_qk per column half so the first
                # energy matmul (which only needs x_qk half 0) starts as soon
                # as the half-0 stats broadcast lands
                for h in range(2):
                    cols = slice(512 * h, 512 * (h + 1))
                    nc.vector.tensor_mul(img[:, cols], img[:, cols],
                                         alphaB[:, cols].bitcast(F32R))
                    nc.vector.tensor_add(img[:, cols], img[:, cols],
                                         beta2B[:, cols].bitcast(F32R))
                    xg_ps = paux.tile([V, 512], fp32, tag="aux",
                                      name=f"xqp{b}_{h}")
                    nc.tensor.matmul(xg_ps[:, :], qkT_sb[:], img[:, cols],
                                     start=True, stop=True)
                    nc.vector.tensor_copy(xg_sb[:, cols], xg_ps[:, :])
            else:
                nc.vector.tensor_mul(img[:], img[:], alphaB[:].bitcast(F32R))
                nc.gpsimd.tensor_add(img[:], img[:], beta2B[:].bitcast(F32R))

                # x_qk = qk_w @ img  (natural [V, D] layout)
                for h in range(2):
                    cols = slice(512 * h, 512 * (h + 1))
                    xg_ps = paux.tile([V, 512], fp32, tag="aux",
                                      name=f"xqp{b}_{h}")
                    nc.tensor.matmul(xg_ps[:, :], qkT_sb[:], img[:, cols],
                                     start=True, stop=True)
                    nc.vector.tensor_copy(xg_sb[:, cols], xg_ps[:, :])

            state[b] = (img, xg_sb, None,
                        py_pool.tile([33, D], fp32, tag="y", name=f"py{b}"))

        def prepare_xv(b):
            # x_v^T (+bias) for all d-blocks: [128, 8] per block -> [128, 64].
            # Emitted later than prepare(): xaug only needs x_v after the
            # first exp of batch b, so this stays off the prep critical path.
            img, xg_sb, _, py = state[b]
            xv_ps = paux.tile([128, V * NBLK], fp32, tag="aux", name=f"xvp{b}")
            for k in range(NBLK):
                cols = slice(V * k, V * (k + 1))
                dblk = slice(128 * k, 128 * (k + 1))
                nc.tensor.matmul(xv_ps[:, cols], img[:, dblk], vwT_sb[:],
                                 start=True, stop=False)
                nc.tensor.matmul(xv_ps[:, cols], ones_row[:], vb_sb[:],
                                 start=False, stop=True)
            xv_sb = xvp.tile([128, V * NBLK], fp32, tag="xv", name=f"xv{b}")
            nc.vector.tensor_copy(xv_sb[:], xv_ps[:])
            state[b] = (img, xg_sb, xv_sb, py)

        def blocks(b, ks):
            img, xg_sb, xv_sb, py = state[b]
            for k in ks:
                dblk = slice(128 * k, 128 * (k + 1))
                pe = pe_pool.tile([128, D], fp32, tag="pe", name=f"pe{b}_{k}")
                for h in range(2):
                    cols = slice(512 * h, 512 * (h + 1))
                    nc.tensor.matmul(pe[:, cols], xg_sb[:, dblk],
                                     xg_sb[:, cols],
                                     start=True, stop=True)
                esb = esbp.tile([128, D], F32R, tag="esb", name=f"esb{b}_{k}")
                rowsum = smallp.tile([128, 1], fp32, tag="rs", name=f"rs{b}_{k}")
                nc.scalar.activation(esb[:], pe[:, :], EXP, accum_out=rowsum[:])
                xaug = xaug_ring[(b * NBLK + k) % 3]
                with nc.allow_low_precision(reason="f32r recip, 4e-4 budget"):
                    nc.vector.reciprocal(xaug[:, 32:33], rowsum[:])
                nc.vector.tensor_scalar_mul(xaug[:, 0:V],
                                            xv_sb[:, V * k : V * (k + 1)],
                                            xaug[:, 32:33].bitcast(fp32))
                for h in range(2):
                    cols = slice(512 * h, 512 * (h + 1))
                    nc.tensor.matmul(py[:, cols], xaug[:], esb[:, cols],
                                     start=(k == 0), stop=(k == NBLK - 1))

        def finalize(b):
            # column renorm + residual, in two column halves so the store
            # of half 0 overlaps the math of half 1. Partition bases of
            # reads must be 32-aligned, hence colsum lives at Y row 32.
            img, xg_sb, xv_sb, py = state.pop(b)
            s_tmp = finp.tile([1, D], fp32, tag="stmp", name=f"st{b}")
            s_sb = finp.tile([1, D], fp32, tag="ssb", name=f"ss{b}")
            sB = finp.tile([V, D], fp32, tag="sB", name=f"sB{b}")
            osb = finp.tile([V, D], fp32, tag="osb", name=f"osb{b}")
            if b < BPC - 1:
                nc.vector.tensor_scalar_add(s_tmp[:], py[32:33, :], 1e-9)
                nc.vector.reciprocal_approx_fast(s_sb[:], s_tmp[:])
                nc.gpsimd.partition_broadcast(sB[:], s_sb[:])
                nc.vector.tensor_tensor(osb[:], py[0:V, :], sB[:], op=MULT)
                if b == 0:
                    nc.vector.tensor_add(osb[:], osb[:], img[:].bitcast(fp32))
                else:
                    nc.gpsimd.tensor_add(osb[:], osb[:], img[:].bitcast(fp32))
                nc.sync.dma_start(out_ap[V * b : V * (b + 1), :], osb[:])
            else:
                # last batch: nothing runs after this chain, so split it into
                # column halves interleaved across DVE/Pool to halve the
                # serial tail, and keep the residual add on DVE
                halves = [slice(0, 512), slice(512, 1024)]
                for c in halves:
                    nc.vector.tensor_scalar_add(s_tmp[:, c], py[32:33, c], 1e-9)
                    nc.vector.reciprocal_approx_fast(s_sb[:, c], s_tmp[:, c])
                    nc.gpsimd.partition_broadcast(sB[:, c], s_sb[:, c])
                for c in halves:
                    nc.vector.tensor_tensor(osb[:, c], py[0:V, c],
                                            sB[:, c], op=MULT)
                    nc.vector.tensor_add(osb[:, c], osb[:, c],
                                         img[:, c].bitcast(fp32))
                    nc.sync.dma_start(out_ap[V * b : V * (b + 1), c],
                                      osb[:, c])

        prepare(0)
        prepare_xv(0)
        XV_AT = 4
        for b in range(BPC):
            blocks(b, range(0, 1))
            if b + 1 < BPC:
                prepare(b + 1)
            blocks(b, range(1, XV_AT))
            if b + 1 < BPC:
                prepare_xv(b + 1)
            blocks(b, range(XV_AT, NBLK))
            finalize(b)

    nc.compile()
    return nc


def _get():
    global _BUILT
    if _BUILT is None:
        _BUILT = _build_program()
    return _BUILT


def _make_in_maps(inputs):
    feat = np.ascontiguousarray(np.asarray(inputs["feat"], dtype=np.float32))
    gamma = np.asarray(inputs["bn_gamma"], dtype=np.float32).reshape(1, D)
    beta = np.asarray(inputs["bn_beta"], dtype=np.float32).reshape(1, D)
    qk = np.ascontiguousarray(np.asarray(inputs["qk_weight"], dtype=np.float32))
    vw = np.ascontiguousarray(np.asarray(inputs["v_weight"], dtype=np.float32))
    vb = np.asarray(inputs["v_bias"], dtype=np.float32).reshape(1, V)
    full = np.ascontiguousarray(feat.reshape(ROWS, D))
    in_maps = []
    for c in range(NCORES):
        shard = np.ascontiguousarray(
            feat[BPC * c : BPC * (c + 1)].reshape(SHARD_ROWS, D))
        in_maps.append({
            "feat_full": full,
            "feat_shard": shard,
            "gamma": gamma,
            "beta": beta,
            "qk_w": qk,
            "v_w": vw,
            "v_bias": vb,
        })
    return in_maps


def _run(inputs, **kw):
    from concourse.bass_utils import run_bass_kernel_spmd
    nc = _get()
    res = run_bass_kernel_spmd(nc, _make_in_maps(inputs),
                               core_ids=list(range(NCORES)), **kw)
    out = np.concatenate(
        [res.results[c]["out"].reshape(BPC, V, D) for c in range(NCORES)],
        axis=0)
    return out, res


def kernel(**inputs) -> np.ndarray:
    out, _ = _run(inputs)
    return out


def run_profiled(inputs, **kw):
    return _run(inputs, trace=True, **kw)



# revision 12
# speedup vs baseline: 1.0197x; 1.0197x over previous
"""Trainium2 Bass kernel for nn_Attention_8358006358422.

Reference computation (B=64, V=8, D=1024):
    BN over all B*V rows per feature d -> img
    x_qk = qk_w @ img ; x_v = v_w @ img + bias
    energy[b] = x_qk[b]^T x_qk[b]  (D x D, contraction over V)
    att = softmax(energy, -1); att /= (1e-9 + sum(att, axis=1))
    out = img + x_v @ att

Kernel strategy (8 NeuronCores, data-parallel over B, 8 batches/core).
The 64 exp instructions on the ACT engine ([128,1024] each, plus the
rowsum-accumulator read) are the hard floor (~78us); everything else is
organized to hide under a saturated ACT pipeline:

  * 4 batches are packed per [128, 1024] tile at partition offsets 32*i
    (quads).  A host-built block-diagonal qk^T [128,128] computes all 4
    batches' un-normalized x_qk in one matmul pair; BN folds in as
      xg = alpha (.) (qk@feat) + beta2 * qkrowsum
    so the first exp never waits for a separate img pass.
  * Energy matmuls are row-tiled (tile_position=(32i,0)): lhsT/rhs are
    8-partition slices at 32-aligned bases, out at PSUM base 0.
  * softmax folded algebraically (as the v1 kernel):
        E = exp(energy); rowsum via the ACT accumulator
        Y = [xv^T/r ; 1/r]^T @ E, col renorm + residual at the end.
  * Y accumulates batch-serial into py [33, 1024] (colsum at the
    32-aligned row 32); py double-buffered so finalize(b) overlaps
    batch b+1.
  * Finalize: reciprocal straight off the PSUM colsum row (the 1e-9
    guard is numerically irrelevant), ksa mul on DVE, residual add on
    GPSIMD, per-batch img tiles produced by background SBUF->SBUF DMA.
"""

import sys
import numpy as np

sys.path.insert(0, "/opt/trn_rl_repo")

B, V, D = 64, 8, 1024
NCORES = 8
BPC = B // NCORES          # batches per core
ROWS = B * V               # 512 BN rows
SHARD_ROWS = BPC * V       # 64
NBLK = D // 128            # 8 d-blocks of 128
BN_EPS = 1e-5

_BUILT = None


def _build_program():
    import concourse.mybir as mybir
    import concourse.tile as tile
    from concourse import bacc
    from contextlib import ExitStack

    fp32 = mybir.dt.float32
    F32R = mybir.dt.float32r
    MULT = mybir.AluOpType.mult
    ADD = mybir.AluOpType.add
    SUB = mybir.AluOpType.subtract
    EXP = mybir.ActivationFunctionType.Exp
    LN = mybir.ActivationFunctionType.Ln

    nc = bacc.Bacc(
        "TRN2",
        target_bir_lowering=False,
        debug=False,
        enable_asserts=False,
        num_devices=NCORES,
    )

    # ---- DRAM I/O ----
    feat_full = nc.dram_tensor("feat_full", [ROWS, D], F32R, kind="ExternalInput")
    feat_shard = nc.dram_tensor("feat_shard", [SHARD_ROWS, D], F32R,
                                kind="ExternalInput")
    gamma_d = nc.dram_tensor("gamma", [1, D], fp32, kind="ExternalInput")
    beta_d = nc.dram_tensor("beta", [1, D], fp32, kind="ExternalInput")
    qkbd_d = nc.dram_tensor("qk_bd", [128, 128], F32R, kind="ExternalInput")
    qrow_d = nc.dram_tensor("qk_rowsum", [128, 1], fp32, kind="ExternalInput")
    vwT4_d = nc.dram_tensor("vwT4", [128, V], F32R, kind="ExternalInput")
    vb_d = nc.dram_tensor("v_bias", [1, 256], fp32, kind="ExternalInput")
    out_d = nc.dram_tensor("out", [SHARD_ROWS, D], fp32, kind="ExternalOutput")
    import os
    DBG = bool(int(os.environ.get("KDBG", "0")))
    if DBG:
        dbg_ab = nc.dram_tensor("dbg_ab", [2, D], fp32, kind="ExternalOutput")
        dbg_xg = nc.dram_tensor("dbg_xg", [128, D], fp32, kind="ExternalOutput")
        dbg_img = nc.dram_tensor("dbg_img", [128, D], fp32, kind="ExternalOutput")
        dbg_xvt = nc.dram_tensor("dbg_xvt", [128, 256], fp32, kind="ExternalOutput")

    with tile.TileContext(nc) as tc, ExitStack() as ctx:
        const = ctx.enter_context(tc.tile_pool(name="const", bufs=1))
        ftp = ctx.enter_context(tc.tile_pool(name="ftp", bufs=2))
        sqp = ctx.enter_context(tc.tile_pool(name="sqp", bufs=2))
        quadp = ctx.enter_context(tc.tile_pool(name="quadp", bufs=2))
        esbp = ctx.enter_context(tc.tile_pool(name="esbp", bufs=4))
        smallp = ctx.enter_context(tc.tile_pool(name="smallp", bufs=6))
        finp = ctx.enter_context(tc.tile_pool(name="finp", bufs=2))
        imgbp = ctx.enter_context(tc.tile_pool(name="imgbp", bufs=2))

        # PSUM: pe_pool 2x2 banks (energy / fq / xv), py_pool 2x2 banks
        # (Y accumulators / BN stats)
        pe_pool = ctx.enter_context(tc.tile_pool(name="pe", bufs=2, space="PSUM"))
        py_pool = ctx.enter_context(tc.tile_pool(name="py", bufs=2, space="PSUM"))

        # ---- constants / weights ----
        ones_col = const.tile([128, 1], fp32)
        nc.vector.memset(ones_col[:], 1.0)
        ones_col_r = const.tile([128, 1], F32R)
        nc.vector.tensor_scalar_mul(ones_col_r[:], ones_col[:], 1.0)

        # ---- feat_full stream + BN statistics ----
        qkbd_sb = const.tile([128, 128], F32R)
        nc.sync.dma_start(qkbd_sb[:], qkbd_d[:, :])
        qrow_sb = const.tile([128, 1], fp32)
        nc.sync.dma_start(qrow_sb[:], qrow_d[:, :])
        gamma_sb = const.tile([1, D], fp32)
        nc.sync.dma_start(gamma_sb[:], gamma_d[:, :])
        beta_sb = const.tile([1, D], fp32)
        nc.sync.dma_start(beta_sb[:], beta_d[:, :])
        vwT4_sb = const.tile([128, V], F32R)
        nc.sync.dma_start(vwT4_sb[:], vwT4_d[:, :])
        vb_sb = const.tile([1, 256], fp32)
        nc.sync.dma_start(vb_sb[:], vb_d[:, :])
        vbB = const.tile([128, 256], fp32)
        nc.gpsimd.partition_broadcast(vbB[:], vb_sb[:])

        # raw shard rows for quad 0 (batches 0-3) at partitions 32i+v
        img4 = [None, None]
        img4[0] = quadp.tile([128, D], F32R, tag="img4", name="img4_0")
        for i in range(4):
            nc.sync.dma_start(img4[0][32 * i:32 * i + V, :],
                              feat_shard[V * i: V * (i + 1), :])

        sum_ps = py_pool.tile([1, D], fp32, tag="py", name="sum_ps")
        sq_ps = py_pool.tile([1, D], fp32, tag="py", name="sq_ps")
        for r in range(4):
            ft = ftp.tile([128, D], F32R, tag="ft", name=f"ft{r}")
            nc.sync.dma_start(ft[:], feat_full[128 * r: 128 * (r + 1), :])
            sq = sqp.tile([128, D], F32R, tag="sq", name=f"sq{r}")
            nc.vector.tensor_mul(sq[:], ft[:], ft[:])
            st = r == 0
            sp = r == 3
            for h in range(2):
                cols = slice(512 * h, 512 * (h + 1))
                nc.tensor.matmul(sum_ps[0:1, cols], ones_col_r[:], ft[:, cols],
                                 start=st, stop=sp)
                nc.tensor.matmul(sq_ps[0:1, cols], ones_col_r[:], sq[:, cols],
                                 start=st, stop=sp)

        # ---- BN chain in column halves:
        #   alpha = gamma * (var+eps)^-1/2,  beta2 = beta - mean*alpha
        # via rstd = exp(-0.5*ln(u/512 + eps)), u = sq - sum^2/512
        mean_sb = const.tile([1, D], fp32)
        t1 = const.tile([1, D], fp32)
        u = const.tile([1, D], fp32)
        rv = const.tile([1, D], fp32)
        rstd = const.tile([1, D], fp32)
        alpha_row = const.tile([1, D], fp32)
        beta2_row = const.tile([1, D], fp32)
        malpha = t1  # reuse: t1's last read is in u's stt
        alphaB = const.tile([128, D], fp32)
        beta2B = const.tile([128, D], fp32)
        for h in range(2):
            c = slice(512 * h, 512 * (h + 1))
            nc.vector.tensor_scalar_mul(mean_sb[:, c], sum_ps[0:1, c],
                                        1.0 / ROWS)
            nc.vector.scalar_tensor_tensor(t1[:, c], mean_sb[:, c],
                                           float(ROWS), mean_sb[:, c],
                                           op0=MULT, op1=MULT)
            # u = (sq - 512*eps_neg) - sum^2/512 with the BN eps pre-folded
            nc.vector.scalar_tensor_tensor(u[:, c], sq_ps[0:1, c],
                                           -float(ROWS) * BN_EPS, t1[:, c],
                                           op0=SUB, op1=SUB)
            nc.scalar.activation(rv[:, c], u[:, c], LN, scale=1.0 / ROWS)
            nc.scalar.activation(rstd[:, c], rv[:, c], EXP, scale=-0.5)
            nc.vector.tensor_mul(alpha_row[:, c], gamma_sb[:, c], rstd[:, c])
            nc.vector.tensor_mul(malpha[:, c], mean_sb[:, c],
                                 alpha_row[:, c])
            nc.vector.tensor_sub(beta2_row[:, c], beta_sb[:, c], malpha[:, c])
            nc.gpsimd.partition_broadcast(alphaB[:, c], alpha_row[:, c])
            nc.gpsimd.partition_broadcast(beta2B[:, c], beta2_row[:, c])

        out_ap = out_d[:, :]

        xaug_ring = []
        for t in range(3):
            xa = const.tile([128, 33], F32R, name=f"xaug_ring{t}")
            nc.vector.memset(xa[:].bitcast(fp32), 0.0)
            xaug_ring.append(xa)

        # per-quad state: xg4 (normalized x_qk, packed), xvT (x_v^T packed
        # per batch: cols 64*i + 8*k + v), per-batch img tiles
        xg4 = [None, None]
        xvT = [None, None]
        img_b = [None] * BPC

        def prep_quad(q):
            # fq = blockdiag(qk) @ raw_feat   (all 4 batches, one mm pair)
            fq_ps = pe_pool.tile([128, D], fp32, tag="pe", name=f"fq{q}")
            for h in range(2):
                cols = slice(512 * h, 512 * (h + 1))
                nc.tensor.matmul(fq_ps[:, cols], qkbd_sb[:],
                                 img4[q][:, cols], start=True, stop=True)
            # xg = alpha (.) fq + beta2 * qkrowsum   (2 DVE ops)
            xg = quadp.tile([128, D], F32R, tag="xg4", name=f"xg4_{q}")
            for h in range(2):
                c = slice(512 * h, 512 * (h + 1))
                nc.vector.tensor_tensor(
                    xg[:, c], fq_ps[:, c].bitcast(F32R),
                    alphaB[:, c].bitcast(F32R), op=MULT)
                nc.vector.scalar_tensor_tensor(
                    xg[:, c], beta2B[:, c].bitcast(F32R),
                    qrow_sb[:], xg[:, c], op0=MULT, op1=ADD)
            xg4[q] = xg

            # BN-apply in place on img4 (residual source):
            # img = alpha (.) raw + beta2
            for h in range(2):
                c = slice(512 * h, 512 * (h + 1))
                nc.vector.tensor_mul(img4[q][:, c], img4[q][:, c],
                                     alphaB[:, c].bitcast(F32R))
                nc.vector.tensor_add(img4[q][:, c], img4[q][:, c],
                                     beta2B[:, c].bitcast(F32R))

            # per-batch img copies for the residual add (partition move via
            # SBUF->SBUF DMA; off the critical path)
            for i in range(4):
                b = 4 * q + i
                imt = imgbp.tile([V, D], fp32, tag=f"img_b{b % 4}",
                                 name=f"img_b{b}")
                nc.sync.dma_start(imt[:],
                                  img4[q][32 * i:32 * i + V, :].bitcast(fp32))
                img_b[b] = imt

            # xvT: per (i, k): [128, 8] = img_slice^T @ vwT (+ bias)
            xv_ps = pe_pool.tile([128, 256], fp32, tag="pe", name=f"xv{q}")
            for i in range(4):
                base = 32 * i
                for k in range(NBLK):
                    cols = slice(64 * i + V * k, 64 * i + V * (k + 1))
                    dblk = slice(128 * k, 128 * (k + 1))
                    nc.tensor.matmul(xv_ps[:, cols],
                                     img4[q][base:base + V, dblk],
                                     vwT4_sb[base:base + V, :],
                                     start=True, stop=True,
                                     tile_position=(base, 0))
            xv = quadp.tile([128, 256], fp32, tag="xvT", name=f"xvT{q}")
            nc.vector.tensor_add(xv[:], xv_ps[:], vbB[:])
            xvT[q] = xv

        def load_quad1():
            img4[1] = quadp.tile([128, D], F32R, tag="img4", name="img4_1")
            for i in range(4):
                nc.sync.dma_start(img4[1][32 * i:32 * i + V, :],
                                  feat_shard[32 + V * i: 32 + V * (i + 1), :])

        py_of = {}

        def blocks(b, ks):
            q, i = b // 4, b % 4
            base = 32 * i
            if b not in py_of:
                py_of[b] = py_pool.tile([33, D], fp32, tag="py",
                                        name=f"py{b}")
            py = py_of[b]
            for k in ks:
                dblk = slice(128 * k, 128 * (k + 1))
                pe = pe_pool.tile([128, D], fp32, tag="pe", name=f"pe{b}_{k}")
                for h in range(2):
                    cols = slice(512 * h, 512 * (h + 1))
                    nc.tensor.matmul(pe[:, cols], xg4[q][base:base + V, dblk],
                                     xg4[q][base:base + V, cols],
                                     start=True, stop=True,
                                     tile_position=(base, 0))
                esb = esbp.tile([128, D], F32R, tag="esb", name=f"esb{b}_{k}")
                rs = smallp.tile([128, 1], fp32, tag="rs", name=f"rs{b}_{k}")
                nc.scalar.activation(esb[:], pe[:, :], EXP, accum_out=rs[:])
                xaug = xaug_ring[(b * NBLK + k) % 3]
                with nc.allow_low_precision(reason="f32r recip, 4e-4 budget"):
                    nc.vector.reciprocal(xaug[:, 32:33], rs[:])
                nc.vector.tensor_scalar_mul(
                    xaug[:, 0:V], xvT[q][:, 64 * i + V * k:64 * i + V * (k + 1)],
                    xaug[:, 32:33].bitcast(fp32))
                for h in range(2):
                    cols = slice(512 * h, 512 * (h + 1))
                    nc.tensor.matmul(py[0:33, cols], xaug[:], esb[:, cols],
                                     start=(k == 0), stop=(k == NBLK - 1))

        def finalize(b, last=False):
            py = py_of.pop(b)
            if not last:
                s_sb = finp.tile([1, D], fp32, tag="ssb", name=f"ss{b}")
                sB = finp.tile([V, D], fp32, tag="sB", name=f"sB{b}")
                osb = finp.tile([V, D], fp32, tag="osb", name=f"osb{b}")
                # 1e-9 guard dropped: colsum > 0 by construction
                nc.vector.reciprocal(s_sb[:], py[32:33, :])
                nc.gpsimd.partition_broadcast(sB[:], s_sb[:])
                nc.vector.tensor_tensor(osb[:], py[0:V, :], sB[:], op=MULT)
                nc.gpsimd.tensor_add(osb[:], osb[:], img_b[b][:])
                nc.sync.dma_start(out_ap[V * b: V * (b + 1), :], osb[:])
            else:
                # tail: halves pipelined across DVE/Pool/DMA
                s_sb = finp.tile([1, D], fp32, tag="ssb", name=f"ss{b}")
                sB = finp.tile([V, D], fp32, tag="sB", name=f"sB{b}")
                osb = finp.tile([V, D], fp32, tag="osb", name=f"osb{b}")
                for h in range(2):
                    c = slice(512 * h, 512 * (h + 1))
                    nc.vector.reciprocal(s_sb[:, c], py[32:33, c])
                    nc.gpsimd.partition_broadcast(sB[:, c], s_sb[:, c])
                for h in range(2):
                    c = slice(512 * h, 512 * (h + 1))
                    nc.vector.tensor_tensor(osb[:, c], py[0:V, c],
                                            sB[:, c], op=MULT)
                    nc.vector.tensor_add(osb[:, c], osb[:, c],
                                         img_b[b][:, c])
                    nc.sync.dma_start(out_ap[V * b: V * (b + 1), c],
                                      osb[:, c])

        prep_quad(0)
        if DBG:
            nc.sync.dma_start(dbg_ab[0:1, :], alpha_row[:])
            nc.sync.dma_start(dbg_ab[1:2, :], beta2_row[:])
            nc.sync.dma_start(dbg_xg[:, :], xg4[0][:].bitcast(fp32))
            nc.sync.dma_start(dbg_img[:, :], img4[0][:].bitcast(fp32))
            nc.sync.dma_start(dbg_xvt[:, :], xvT[0][:])
        for b in range(BPC):
            if b == 2:
                blocks(b, range(0, 1))
                load_quad1()
                blocks(b, range(1, NBLK))
            elif b == 3:
                blocks(b, range(0, 1))
                prep_quad(1)
                blocks(b, range(1, NBLK))
            else:
                blocks(b, range(0, 2))
                if b > 0:
                    finalize(b - 1)
                blocks(b, range(2, NBLK))
            if b in (2, 3):
                finalize(b - 1)
        finalize(BPC - 1, last=True)

    nc.compile()
    return nc


def _get():
    global _BUILT
    if _BUILT is None:
        _BUILT = _build_program()
    return _BUILT


def _make_in_maps(inputs):
    feat = np.ascontiguousarray(np.asarray(inputs["feat"], dtype=np.float32))
    gamma = np.asarray(inputs["bn_gamma"], dtype=np.float32).reshape(1, D)
    beta = np.asarray(inputs["bn_beta"], dtype=np.float32).reshape(1, D)
    qk = np.ascontiguousarray(np.asarray(inputs["qk_weight"], dtype=np.float32))
    vw = np.ascontiguousarray(np.asarray(inputs["v_weight"], dtype=np.float32))
    vb = np.tile(np.asarray(inputs["v_bias"], dtype=np.float32).reshape(1, V),
                 (1, 32))
    full = np.ascontiguousarray(feat.reshape(ROWS, D))

    # block-diagonal qk^T: rows 32i+u, cols 32i+v = qk[v, u]
    qkbd = np.zeros((128, 128), dtype=np.float32)
    qrow = np.zeros((128, 1), dtype=np.float32)
    vwT4 = np.zeros((128, V), dtype=np.float32)
    for i in range(4):
        qkbd[32 * i:32 * i + V, 32 * i:32 * i + V] = qk.T
        qrow[32 * i:32 * i + V, 0] = qk.sum(axis=1)
        vwT4[32 * i:32 * i + V, :] = vw.T
    in_maps = []
    for c in range(NCORES):
        shard = np.ascontiguousarray(
            feat[BPC * c: BPC * (c + 1)].reshape(SHARD_ROWS, D))
        in_maps.append({
            "feat_full": full,
            "feat_shard": shard,
            "gamma": gamma,
            "beta": beta,
            "qk_bd": qkbd,
            "qk_rowsum": qrow,
            "vwT4": vwT4,
            "v_bias": vb,
        })
    return in_maps


def _run(inputs, **kw):
    from concourse.bass_utils import run_bass_kernel_spmd
    nc = _get()
    res = run_bass_kernel_spmd(nc, _make_in_maps(inputs),
                               core_ids=list(range(NCORES)), **kw)
    out = np.concatenate(
        [res.results[c]["out"].reshape(BPC, V, D) for c in range(NCORES)],
        axis=0)
    return out, res


def kernel(**inputs) -> np.ndarray:
    out, _ = _run(inputs)
    return out


def run_profiled(inputs, **kw):
    return _run(inputs, trace=True, **kw)


# revision 15
# speedup vs baseline: 1.0500x; 1.0297x over previous
"""Trainium2 Bass kernel for nn_Attention_8358006358422.

Reference computation (B=64, V=8, D=1024):
    BN over all B*V rows per feature d -> img
    x_qk = qk_w @ img ; x_v = v_w @ img + bias
    energy[b] = x_qk[b]^T x_qk[b]  (D x D, contraction over V)
    att = softmax(energy, -1); att /= (1e-9 + sum(att, axis=1))
    out = img + x_v @ att

Kernel strategy (8 NeuronCores, data-parallel over B, 8 batches/core).
The 64 exp instructions on the ACT engine ([128,1024] each, plus the
rowsum-accumulator read) are the hard floor (~78us); everything else is
organized to hide under a saturated ACT pipeline:

  * 4 batches are packed per [128, 1024] tile at partition offsets 32*i
    (quads).  A host-built block-diagonal qk^T [128,128] computes all 4
    batches' un-normalized x_qk in one matmul pair; BN folds in as
      xg = alpha (.) (qk@feat) + beta2 * qkrowsum
    so the first exp never waits for a separate img pass.
  * Energy matmuls are row-tiled (tile_position=(32i,0)): lhsT/rhs are
    8-partition slices at 32-aligned bases, out at PSUM base 0.
  * softmax folded algebraically (as the v1 kernel):
        E = exp(energy); rowsum via the ACT accumulator
        Y = [xv^T/r ; 1/r]^T @ E, col renorm + residual at the end.
  * Y accumulates batch-serial into py [33, 1024] (colsum at the
    32-aligned row 32); py double-buffered so finalize(b) overlaps
    batch b+1.
  * Finalize: reciprocal straight off the PSUM colsum row (the 1e-9
    guard is numerically irrelevant), ksa mul on DVE, residual add on
    GPSIMD, per-batch img tiles produced by background SBUF->SBUF DMA.
"""

import sys
import numpy as np

sys.path.insert(0, "/opt/trn_rl_repo")

B, V, D = 64, 8, 1024
NCORES = 8
BPC = B // NCORES          # batches per core
ROWS = B * V               # 512 BN rows
SHARD_ROWS = BPC * V       # 64
NBLK = D // 128            # 8 d-blocks of 128
BN_EPS = 1e-5

_BUILT = None


def _build_program():
    import concourse.mybir as mybir
    import concourse.tile as tile
    from concourse import bacc
    from contextlib import ExitStack

    fp32 = mybir.dt.float32
    F32R = mybir.dt.float32r
    MULT = mybir.AluOpType.mult
    ADD = mybir.AluOpType.add
    SUB = mybir.AluOpType.subtract
    EXP = mybir.ActivationFunctionType.Exp
    LN = mybir.ActivationFunctionType.Ln

    nc = bacc.Bacc(
        "TRN2",
        target_bir_lowering=False,
        debug=False,
        enable_asserts=False,
        num_devices=NCORES,
    )

    # ---- DRAM I/O ----
    feat_full = nc.dram_tensor("feat_full", [ROWS, D], F32R, kind="ExternalInput")
    feat_shard = nc.dram_tensor("feat_shard", [SHARD_ROWS, D], F32R,
                                kind="ExternalInput")
    gamma_d = nc.dram_tensor("gamma", [1, D], fp32, kind="ExternalInput")
    beta_d = nc.dram_tensor("beta", [1, D], fp32, kind="ExternalInput")
    qkbd_d = nc.dram_tensor("qk_bd", [128, 128], F32R, kind="ExternalInput")
    qrow_d = nc.dram_tensor("qk_rowsum", [128, 1], fp32, kind="ExternalInput")
    vwT4_d = nc.dram_tensor("vwT4", [128, V], F32R, kind="ExternalInput")
    vb_d = nc.dram_tensor("v_bias", [1, 256], fp32, kind="ExternalInput")
    out_d = nc.dram_tensor("out", [SHARD_ROWS, D], fp32, kind="ExternalOutput")
    import os
    DBG = bool(int(os.environ.get("KDBG", "0")))
    if DBG:
        dbg_ab = nc.dram_tensor("dbg_ab", [2, D], fp32, kind="ExternalOutput")
        dbg_xg = nc.dram_tensor("dbg_xg", [128, D], fp32, kind="ExternalOutput")
        dbg_img = nc.dram_tensor("dbg_img", [128, D], fp32, kind="ExternalOutput")
        dbg_xvt = nc.dram_tensor("dbg_xvt", [128, 256], fp32, kind="ExternalOutput")

    with tile.TileContext(nc) as tc, ExitStack() as ctx:
        const = ctx.enter_context(tc.tile_pool(name="const", bufs=1))
        ftp = ctx.enter_context(tc.tile_pool(name="ftp", bufs=4))
        sqp = ctx.enter_context(tc.tile_pool(name="sqp", bufs=2))
        quadp = ctx.enter_context(tc.tile_pool(name="quadp", bufs=2))
        esbp = ctx.enter_context(tc.tile_pool(name="esbp", bufs=4))
        smallp = ctx.enter_context(tc.tile_pool(name="smallp", bufs=6))
        finp = ctx.enter_context(tc.tile_pool(name="finp", bufs=2))
        imgbp = ctx.enter_context(tc.tile_pool(name="imgbp", bufs=2))

        # PSUM: pe_pool 2x2 banks (energy / fq / xv), py_pool 2x2 banks
        # (Y accumulators / BN stats)
        pe_pool = ctx.enter_context(tc.tile_pool(name="pe", bufs=2, space="PSUM"))
        py_pool = ctx.enter_context(tc.tile_pool(name="py", bufs=2, space="PSUM"))

        # ---- constants / weights ----
        ones_col = const.tile([128, 1], fp32)
        nc.vector.memset(ones_col[:], 1.0)
        ones_col_r = const.tile([128, 1], F32R)
        nc.vector.tensor_scalar_mul(ones_col_r[:], ones_col[:], 1.0)

        # ---- DMA order: quad-0 shard + qk first (fq path), then ft by
        # column halves so BN-stats half 0 completes early ----
        img4 = [None, None]
        img4[0] = quadp.tile([128, D], F32R, tag="img4", name="img4_0")
        for i in range(4):
            nc.sync.dma_start(img4[0][32 * i:32 * i + V, :],
                              feat_shard[V * i: V * (i + 1), :])
        qkbd_sb = const.tile([128, 128], F32R)
        nc.sync.dma_start(qkbd_sb[:], qkbd_d[:, :])
        qrow_sb = const.tile([128, 1], fp32)
        nc.sync.dma_start(qrow_sb[:], qrow_d[:, :])
        gamma_sb = const.tile([1, D], fp32)
        nc.sync.dma_start(gamma_sb[:], gamma_d[:, :])
        beta_sb = const.tile([1, D], fp32)
        nc.sync.dma_start(beta_sb[:], beta_d[:, :])

        ft_tiles = []
        for r in range(4):
            ft = ftp.tile([128, D], F32R, tag="ft", name=f"ft{r}")
            ft_tiles.append(ft)
        for h in range(2):
            for r in range(4):
                cols = slice(512 * h, 512 * (h + 1))
                nc.sync.dma_start(ft_tiles[r][:, cols],
                                  feat_full[128 * r: 128 * (r + 1), cols])

        vwT4_sb = const.tile([128, V], F32R)
        nc.sync.dma_start(vwT4_sb[:], vwT4_d[:, :])
        vb_sb = const.tile([1, 256], fp32)
        nc.sync.dma_start(vb_sb[:], vb_d[:, :])
        vbB = const.tile([128, 256], fp32)
        nc.gpsimd.partition_broadcast(vbB[:], vb_sb[:])

        sum_ps = py_pool.tile([1, D], fp32, tag="py", name="sum_ps")
        sq_ps = py_pool.tile([1, D], fp32, tag="py", name="sq_ps")
        for h in range(2):
            cols = slice(512 * h, 512 * (h + 1))
            for r in range(4):
                sq = sqp.tile([128, 512], F32R, tag="sq", name=f"sq{r}_{h}")
                nc.vector.tensor_mul(sq[:], ft_tiles[r][:, cols],
                                     ft_tiles[r][:, cols])
                nc.tensor.matmul(sum_ps[0:1, cols], ones_col_r[:],
                                 ft_tiles[r][:, cols],
                                 start=(r == 0), stop=(r == 3))
                nc.tensor.matmul(sq_ps[0:1, cols], ones_col_r[:], sq[:],
                                 start=(r == 0), stop=(r == 3))

        # ---- BN chain per column half:
        #   t1 = sum^2/512 (ACT Square), u = (sq + 512*eps) - t1
        #   rstd = exp(-0.5 * ln(u/512)); alpha = gamma*rstd
        #   beta2 = beta - mean*alpha
        t1 = const.tile([1, D], fp32)
        u = const.tile([1, D], fp32)
        rv = const.tile([1, D], fp32)
        rstd = const.tile([1, D], fp32)
        alpha_row = const.tile([1, D], fp32)
        malpha = const.tile([1, D], fp32)
        beta2_row = const.tile([1, D], fp32)
        alphaB = const.tile([128, D], fp32)
        beta2B = const.tile([128, D], fp32)
        SQUARE = mybir.ActivationFunctionType.Square
        for h in range(2):
            c = slice(512 * h, 512 * (h + 1))
            nc.scalar.activation(t1[:, c], sum_ps[0:1, c], SQUARE,
                                 scale=1.0 / float(ROWS) ** 0.5)
            nc.vector.scalar_tensor_tensor(u[:, c], sq_ps[0:1, c],
                                           -float(ROWS) * BN_EPS, t1[:, c],
                                           op0=SUB, op1=SUB)
            nc.scalar.activation(rv[:, c], u[:, c], LN, scale=1.0 / ROWS)
            nc.scalar.activation(rstd[:, c], rv[:, c], EXP, scale=-0.5)
            nc.vector.tensor_mul(alpha_row[:, c], gamma_sb[:, c], rstd[:, c])
            nc.gpsimd.partition_broadcast(alphaB[:, c], alpha_row[:, c])
            nc.vector.scalar_tensor_tensor(malpha[:, c], sum_ps[0:1, c],
                                           1.0 / ROWS, alpha_row[:, c],
                                           op0=MULT, op1=MULT)
            nc.vector.tensor_sub(beta2_row[:, c], beta_sb[:, c], malpha[:, c])
            nc.gpsimd.partition_broadcast(beta2B[:, c], beta2_row[:, c])

        out_ap = out_d[:, :]

        xaug_ring = []
        for t in range(3):
            xa = const.tile([128, 33], F32R, name=f"xaug_ring{t}")
            nc.vector.memset(xa[:].bitcast(fp32), 0.0)
            xaug_ring.append(xa)

        # per-quad state: xg4 (normalized x_qk, packed), xvT (x_v^T packed
        # per batch: cols 64*i + 8*k + v), per-batch img tiles
        xg4 = [None, None]
        xvT = [None, None]
        img_b = [None] * BPC

        def prep_fq_xg(q):
            # fq = blockdiag(qk) @ raw_feat; xg = alpha (.) fq + beta2*qkrow
            fq_ps = pe_pool.tile([128, D], fp32, tag="pe", name=f"fq{q}")
            for h in range(2):
                cols = slice(512 * h, 512 * (h + 1))
                nc.tensor.matmul(fq_ps[:, cols], qkbd_sb[:],
                                 img4[q][:, cols], start=True, stop=True)
            xg = quadp.tile([128, D], F32R, tag="xg4", name=f"xg4_{q}")
            for h in range(2):
                c = slice(512 * h, 512 * (h + 1))
                nc.vector.tensor_tensor(
                    xg[:, c], fq_ps[:, c].bitcast(F32R),
                    alphaB[:, c].bitcast(F32R), op=MULT)
                nc.vector.scalar_tensor_tensor(
                    xg[:, c], beta2B[:, c].bitcast(F32R),
                    qrow_sb[:], xg[:, c], op0=MULT, op1=ADD)
            xg4[q] = xg

        def prep_img(q):
            # BN-apply in place on img4, then per-batch copies via DMA
            for h in range(2):
                c = slice(512 * h, 512 * (h + 1))
                nc.vector.tensor_mul(img4[q][:, c], img4[q][:, c],
                                     alphaB[:, c].bitcast(F32R))
                nc.vector.tensor_add(img4[q][:, c], img4[q][:, c],
                                     beta2B[:, c].bitcast(F32R))
            for i in range(4):
                b = 4 * q + i
                imt = imgbp.tile([V, D], fp32, tag=f"img_b{b % 4}",
                                 name=f"img_b{b}")
                nc.sync.dma_start(imt[:],
                                  img4[q][32 * i:32 * i + V, :].bitcast(fp32))
                img_b[b] = imt

        def prep_xv(q):
            xv_ps = pe_pool.tile([128, 256], fp32, tag="pe", name=f"xv{q}")
            for i in range(4):
                base = 32 * i
                for k in range(NBLK):
                    cols = slice(64 * i + V * k, 64 * i + V * (k + 1))
                    dblk = slice(128 * k, 128 * (k + 1))
                    nc.tensor.matmul(xv_ps[:, cols],
                                     img4[q][base:base + V, dblk],
                                     vwT4_sb[base:base + V, :],
                                     start=True, stop=True,
                                     tile_position=(base, 0))
            xv = quadp.tile([128, 256], fp32, tag="xvT", name=f"xvT{q}")
            nc.vector.tensor_add(xv[:], xv_ps[:], vbB[:])
            xvT[q] = xv

        def load_quad1():
            img4[1] = quadp.tile([128, D], F32R, tag="img4", name="img4_1")
            for i in range(4):
                nc.sync.dma_start(img4[1][32 * i:32 * i + V, :],
                                  feat_shard[32 + V * i: 32 + V * (i + 1), :])

        py_of = {}
        ENERGY = {}

        def emit_energy(b, k):
            q, i = b // 4, b % 4
            base = 32 * i
            dblk = slice(128 * k, 128 * (k + 1))
            pe = pe_pool.tile([128, D], fp32, tag="pe", name=f"pe{b}_{k}")
            for h in range(2):
                cols = slice(512 * h, 512 * (h + 1))
                nc.tensor.matmul(pe[:, cols], xg4[q][base:base + V, dblk],
                                 xg4[q][base:base + V, cols],
                                 start=True, stop=True,
                                 tile_position=(base, 0))
            ENERGY[(b, k)] = pe

        def emit_exp(b, k):
            pe = ENERGY.pop((b, k))
            esb = esbp.tile([128, D], F32R, tag="esb", name=f"esb{b}_{k}")
            rs = smallp.tile([128, 1], fp32, tag="rs", name=f"rs{b}_{k}")
            nc.scalar.activation(esb[:], pe[:, :], EXP, accum_out=rs[:])
            return esb, rs

        def emit_y(b, k, esb, rs):
            q, i = b // 4, b % 4
            if b not in py_of:
                py_of[b] = py_pool.tile([33, D], fp32, tag="py", name=f"py{b}")
            py = py_of[b]
            xaug = xaug_ring[(b * NBLK + k) % 3]
            with nc.allow_low_precision(reason="f32r recip, 4e-4 budget"):
                nc.vector.reciprocal(xaug[:, 32:33], rs[:])
            nc.vector.tensor_scalar_mul(
                xaug[:, 0:V],
                xvT[q][:, 64 * i + V * k: 64 * i + V * (k + 1)],
                xaug[:, 32:33].bitcast(fp32))
            for h in range(2):
                cols = slice(512 * h, 512 * (h + 1))
                nc.tensor.matmul(py[0:33, cols], xaug[:], esb[:, cols],
                                 start=(k == 0), stop=(k == NBLK - 1))

        def finalize_a(b):
            # reciprocal of the colsum row + partition broadcast
            py = py_of[b]
            s_sb = finp.tile([1, D], fp32, tag="ssb", name=f"ss{b}")
            sB = finp.tile([V, D], fp32, tag="sB", name=f"sB{b}")
            # 1e-9 guard dropped: colsum > 0 by construction
            nc.vector.reciprocal(s_sb[:], py[32:33, :])
            nc.gpsimd.partition_broadcast(sB[:], s_sb[:])
            return sB

        def finalize_b(b, sB, pool_add=True):
            py = py_of.pop(b)
            osb = finp.tile([V, D], fp32, tag="osb", name=f"osb{b}")
            nc.vector.tensor_tensor(osb[:], py[0:V, :], sB[:], op=MULT)
            if pool_add:
                nc.gpsimd.tensor_add(osb[:], osb[:], img_b[b][:])
            else:
                nc.vector.tensor_add(osb[:], osb[:], img_b[b][:])
            nc.sync.dma_start(out_ap[V * b: V * (b + 1), :], osb[:])

        def finalize_tail(b):
            # last batch: quarters pipelined across DVE/Pool/DMA
            py = py_of.pop(b)
            s_sb = finp.tile([1, D], fp32, tag="ssb", name=f"ss{b}")
            sB = finp.tile([V, D], fp32, tag="sB", name=f"sB{b}")
            osb = finp.tile([V, D], fp32, tag="osb", name=f"osb{b}")
            NQ = 4
            W = D // NQ
            for hq in range(NQ):
                c = slice(W * hq, W * (hq + 1))
                nc.vector.reciprocal(s_sb[:, c], py[32:33, c])
                nc.gpsimd.partition_broadcast(sB[:, c], s_sb[:, c])
                nc.vector.tensor_tensor(osb[:, c], py[0:V, c], sB[:, c],
                                        op=MULT)
                nc.vector.tensor_add(osb[:, c], osb[:, c], img_b[b][:, c])
                nc.sync.dma_start(out_ap[V * b: V * (b + 1), c], osb[:, c])

        # ---- pipelined main loop: 2-block energy lookahead keeps the PE
        # FIFO feeding ACT; Y matmuls are emitted after the lookahead so a
        # late xaug never stalls the next energy; prep/finalize work is
        # emitted in small chunks at points where its inputs are ready ----
        prep_fq_xg(0)
        prep_img(0)
        prep_xv(0)
        if DBG:
            nc.sync.dma_start(dbg_ab[0:1, :], alpha_row[:])
            nc.sync.dma_start(dbg_ab[1:2, :], beta2_row[:])
            nc.sync.dma_start(dbg_xg[:, :], xg4[0][:].bitcast(fp32))
            nc.sync.dma_start(dbg_img[:, :], img4[0][:].bitcast(fp32))
            nc.sync.dma_start(dbg_xvt[:, :], xvT[0][:])

        order = [(b, k) for b in range(BPC) for k in range(NBLK)]
        emit_energy(*order[0])
        emit_energy(*order[1])
        sB_of = {}
        for idx, (b, k) in enumerate(order):
            if (b, k) == (1, 0):
                load_quad1()
            if (b, k) == (1, 4):
                prep_fq_xg(1)
            if (b, k) == (2, 0):
                prep_img(1)
            if (b, k) == (2, 4):
                prep_xv(1)
            esb, rs = emit_exp(b, k)
            if idx + 2 < len(order):
                emit_energy(*order[idx + 2])
            emit_y(b, k, esb, rs)
            if k == 1 and b > 0:
                sB_of[b - 1] = finalize_a(b - 1)
            if k == 4 and b > 0:
                finalize_b(b - 1, sB_of.pop(b - 1), pool_add=(b < 7))
        finalize_tail(BPC - 1)

    nc.compile()
    return nc


def _get():
    global _BUILT
    if _BUILT is None:
        _BUILT = _build_program()
    return _BUILT


def _make_in_maps(inputs):
    feat = np.ascontiguousarray(np.asarray(inputs["feat"], dtype=np.float32))
    gamma = np.asarray(inputs["bn_gamma"], dtype=np.float32).reshape(1, D)
    beta = np.asarray(inputs["bn_beta"], dtype=np.float32).reshape(1, D)
    qk = np.ascontiguousarray(np.asarray(inputs["qk_weight"], dtype=np.float32))
    vw = np.ascontiguousarray(np.asarray(inputs["v_weight"], dtype=np.float32))
    vb = np.tile(np.asarray(inputs["v_bias"], dtype=np.float32).reshape(1, V),
                 (1, 32))
    full = np.ascontiguousarray(feat.reshape(ROWS, D))

    # block-diagonal qk^T: rows 32i+u, cols 32i+v = qk[v, u]
    qkbd = np.zeros((128, 128), dtype=np.float32)
    qrow = np.zeros((128, 1), dtype=np.float32)
    vwT4 = np.zeros((128, V), dtype=np.float32)
    for i in range(4):
        qkbd[32 * i:32 * i + V, 32 * i:32 * i + V] = qk.T
        qrow[32 * i:32 * i + V, 0] = qk.sum(axis=1)
        vwT4[32 * i:32 * i + V, :] = vw.T
    in_maps = []
    for c in range(NCORES):
        shard = np.ascontiguousarray(
            feat[BPC * c: BPC * (c + 1)].reshape(SHARD_ROWS, D))
        in_maps.append({
            "feat_full": full,
            "feat_shard": shard,
            "gamma": gamma,
            "beta": beta,
            "qk_bd": qkbd,
            "qk_rowsum": qrow,
            "vwT4": vwT4,
            "v_bias": vb,
        })
    return in_maps


def _run(inputs, **kw):
    from concourse.bass_utils import run_bass_kernel_spmd
    nc = _get()
    res = run_bass_kernel_spmd(nc, _make_in_maps(inputs),
                               core_ids=list(range(NCORES)), **kw)
    out = np.concatenate(
        [res.results[c]["out"].reshape(BPC, V, D) for c in range(NCORES)],
        axis=0)
    return out, res


def kernel(**inputs) -> np.ndarray:
    out, _ = _run(inputs)
    return out


def run_profiled(inputs, **kw):
    return _run(inputs, trace=True, **kw)
